# revision 5
# baseline (speedup 1.0000x reference)
"""Causal self-attention (B=4, T=2048, D=1024, H=16) on 8 trn2 NeuronCores.

Sharding: core = (batch b, head-group g); b in 0..3, g in 0..1.
Each core computes, for its batch and its 8 heads:
  qT = (Wq_g @ x_b^T + bq_g)        [512, 2048]   (bf16, internal)
  kT = (Wk_g @ x_b^T + bk_g)        [512, 2048]   (fp32 out + bf16 internal)
  v  = (x_b @ Wv_g^T + bv_g)        [2048, 512]   (fp32 out + bf16 internal)
  flash-style causal attention per head (no max subtraction; logits are
  O(1) so exp is safe in fp32 PSUM), denominator fused into the PV matmul
  via a ones-column appended to V.
  yT_partial = Wo_g^T-slice projection of the normalized attention output
               [1024, 2048] (fp32 out)
Host: y[b] = (yT_(b,0) + yT_(b,1))^T + bo; k/v reassembled from kT / v.
"""

import os
import sys
import time

import numpy as np

if "/opt/trn_rl_repo" not in sys.path:
    sys.path.insert(0, "/opt/trn_rl_repo")

B, T, D, H = 4, 2048, 1024, 16
HD = 64
NCORES = 8
GF = 512          # features per head-group (8 heads x 64)
NPAIR = 4         # head pairs per group
TC = 512          # moving-dim chunk
NTT = T // 128    # 16 token tiles
NKC = D // 128    # 8 contraction tiles
SCALE = HD ** -0.5

_RUNNER = None


def _split_waits(nc, mybir):
    """The pinned walrus rejects more than one sync wait per instruction.
    Hoist all-but-the-last wait of every instruction into standalone
    single-wait EventSemaphore instructions on the same engine, inserted
    immediately before it."""
    n = 0
    for f in nc.m.functions:
        for bb in f.blocks:
            insts = bb.instructions
            changed = False
            new = []
            for i in insts:
                si = i.sync_info
                if si is not None and si.on_wait and len(si.on_wait) > 1:
                    waits = list(si.on_wait)
                    for w in waits[:-1]:
                        n += 1
                        new.append(mybir.InstEventSemaphore(
                            name=f"wsplit_{n}",
                            engine=i.engine,
                            sync_info=mybir.SyncInfo(on_wait=[w],
                                                     on_update=[]),
                        ))
                    i.sync_info = mybir.SyncInfo(
                        on_wait=[waits[-1]],
                        on_update=list(si.on_update or []),
                    )
                    changed = True
                new.append(i)
            if changed:
                bb.instructions = new
    return n


def _build_bass():
    import concourse.bass as bass
    import concourse.tile as tile
    from concourse import mybir
    from contextlib import ExitStack

    f32 = mybir.dt.float32
    bf16 = mybir.dt.bfloat16
    AF = mybir.ActivationFunctionType

    nc = bass.Bass("TRN2", target_bir_lowering=False, debug=False,
                   num_devices=NCORES)

    xT_d = nc.dram_tensor("xT", [D, T], bf16, kind="ExternalInput").ap()
    wqT_d = nc.dram_tensor("wqT", [D, GF], bf16, kind="ExternalInput").ap()
    wkT_d = nc.dram_tensor("wkT", [D, GF], bf16, kind="ExternalInput").ap()
    wvT_d = nc.dram_tensor("wvT", [D, GF], bf16, kind="ExternalInput").ap()
    woT_d = nc.dram_tensor("woT", [GF, D], bf16, kind="ExternalInput").ap()
    bq_d = nc.dram_tensor("bq", [GF, 1], f32, kind="ExternalInput").ap()
    bk_d = nc.dram_tensor("bk", [GF, 1], f32, kind="ExternalInput").ap()
    bv_d = nc.dram_tensor("bv", [1, GF], f32, kind="ExternalInput").ap()

    yT_d = nc.dram_tensor("yT", [D, T], f32, kind="ExternalOutput").ap()
    kT_d = nc.dram_tensor("kT", [GF, T], f32, kind="ExternalOutput").ap()
    v_d = nc.dram_tensor("v", [T, GF], f32, kind="ExternalOutput").ap()

    with tile.TileContext(nc) as tc, ExitStack() as ctx:
        const = ctx.enter_context(tc.tile_pool(name="const", bufs=1))

        # biases, per f-tile [128, 1]
        bq_sb = []
        bk_sb = []
        for i in range(GF // 128):
            t = const.tile([128, 1], f32, name=f"bq{i}")
            nc.sync.dma_start(t[:, :], bq_d[128 * i:128 * (i + 1), :])
            bq_sb.append(t)
            t = const.tile([128, 1], f32, name=f"bk{i}")
            nc.sync.dma_start(t[:, :], bk_d[128 * i:128 * (i + 1), :])
            bk_sb.append(t)

        # bv broadcast across partitions [128, 512] via a K=1 fp32 matmul
        bv_row = const.tile([1, GF], f32, name="bv_row")
        nc.sync.dma_start(bv_row[:, :], bv_d[:, :])
        ones_col = const.tile([1, 128], f32, name="ones_col")
        nc.vector.memset(ones_col[:, :], 1.0)
        bv_bc = const.tile([128, GF], f32, name="bv_bc")

        # ones row for the reciprocal broadcast matmul
        ones_row = const.tile([1, HD], bf16, name="ones_row")
        nc.vector.memset(ones_row[:, :], 1.0)

        # causal masks for the 4 diagonal offsets: keep (1.0) where
        # i_local - j_local - 128*m >= 0, else 0.0
        masks = []
        for m in range(4):
            mk = const.tile([128, TC], bf16, name=f"mask{m}")
            nc.gpsimd.memset(mk[:, :], 1.0)
            nc.gpsimd.affine_select(
                out=mk[:, :], in_=mk[:, :],
                compare_op=mybir.AluOpType.is_ge,
                fill=0.0, base=-(128 * m),
                pattern=[[1, TC]], channel_multiplier=-1,
            )
            masks.append(mk)

        # ---------------- phase 1: projections ----------------
        xts = []
        wq_sb, wk_sb, wv_sb = [], [], []
        xw = ctx.enter_context(tc.tile_pool(name="xw", bufs=1))
        for c in range(NKC):
            t = xw.tile([128, T], bf16, name=f"xt{c}")
            nc.sync.dma_start(t[:, :], xT_d[128 * c:128 * (c + 1), :])
            xts.append(t)
        for nm, dram, lst in (("wq", wqT_d, wq_sb), ("wk", wkT_d, wk_sb),
                              ("wv", wvT_d, wv_sb)):
            for c in range(NKC):
                t = xw.tile([128, GF], bf16, name=f"{nm}{c}")
                nc.sync.dma_start(t[:, :], dram[128 * c:128 * (c + 1), :])
                lst.append(t)

        qk_pool = ctx.enter_context(tc.tile_pool(name="qk", bufs=1))
        qT = [qk_pool.tile([128, T], bf16, name=f"qT{p}") for p in range(NPAIR)]
        kT = [qk_pool.tile([128, T], bf16, name=f"kT{p}") for p in range(NPAIR)]

        # v_aug tiles: [128, 8 heads, 66] bf16; cols 0..63 data, col 64 ones
        vg_pool = ctx.enter_context(tc.tile_pool(name="vg", bufs=1))
        vts = [vg_pool.tile([128, 8, 66], bf16, name=f"vt{j}")
               for j in range(NTT)]

        stage = ctx.enter_context(tc.tile_pool(name="stage", bufs=4))

        with tc.tile_pool(name="p1psum", bufs=2, space="PSUM") as p1ps:
            bvp = p1ps.tile([128, GF], f32, name="bvp")
            nc.tensor.matmul(bvp[:, :], lhsT=ones_col[:, :],
                             rhs=bv_row[:, :], start=True, stop=True)
            nc.scalar.copy(bv_bc[:, :], bvp[:, :])

            # qT / kT: [f-tile 128, t-chunk 512]
            for p4 in range(NPAIR):
                for ic in range(T // TC):
                    tsl = slice(TC * ic, TC * (ic + 1))
                    ps = p1ps.tile([128, TC], f32, name="psq")
                    for c in range(NKC):
                        nc.tensor.matmul(
                            ps[:, :],
                            lhsT=wq_sb[c][:, 128 * p4:128 * (p4 + 1)],
                            rhs=xts[c][:, tsl],
                            start=(c == 0), stop=(c == NKC - 1),
                        )
                    nc.scalar.activation(qT[p4][:, tsl], ps[:, :],
                                         AF.Identity, bias=bq_sb[p4][:, :])

                    ps2 = p1ps.tile([128, TC], f32, name="psk")
                    for c in range(NKC):
                        nc.tensor.matmul(
                            ps2[:, :],
                            lhsT=wk_sb[c][:, 128 * p4:128 * (p4 + 1)],
                            rhs=xts[c][:, tsl],
                            start=(c == 0), stop=(c == NKC - 1),
                        )
                    kst = stage.tile([128, TC], f32, name="kst")
                    nc.scalar.activation(kst[:, :], ps2[:, :],
                                         AF.Identity, bias=bk_sb[p4][:, :])
                    nc.gpsimd.dma_start(
                        kT_d[128 * p4:128 * (p4 + 1), tsl], kst[:, :])
                    nc.vector.tensor_copy(kT[p4][:, tsl], kst[:, :])

            # v natural layout: [t-tile 128, 512]
            for j in range(NTT):
                ps = p1ps.tile([128, GF], f32, name="psv")
                for c in range(NKC):
                    nc.tensor.matmul(
                        ps[:, :],
                        lhsT=xts[c][:, 128 * j:128 * (j + 1)],
                        rhs=wv_sb[c][:, :],
                        start=(c == 0), stop=(c == NKC - 1),
                    )
                vst = stage.tile([128, GF], f32, name="vst")
                nc.vector.tensor_add(vst[:, :], ps[:, :], bv_bc[:, :])
                nc.gpsimd.dma_start(v_d[128 * j:128 * (j + 1), :], vst[:, :])
                nc.vector.tensor_copy(
                    vts[j][:, :, 0:HD],
                    vst[:, :].rearrange("p (h d) -> p h d", h=8),
                )
                nc.vector.memset(vts[j][:, :, HD:HD + 1], 1.0)

        # ---------------- phase 2: attention ----------------
        outn_pool = ctx.enter_context(tc.tile_pool(name="outn", bufs=1))
        outTn = [outn_pool.tile([128, T], bf16, name=f"outTn{p}")
                 for p in range(NPAIR)]

        rec_pool = ctx.enter_context(tc.tile_pool(name="rec", bufs=4))
        sbbc_pool = ctx.enter_context(tc.tile_pool(name="sbbc", bufs=2))
        pt_pool = ctx.enter_context(tc.tile_pool(name="pt", bufs=4))

        with tc.tile_pool(name="spsum", bufs=2, space="PSUM") as sps_pool, \
             tc.tile_pool(name="opsum", bufs=1, space="PSUM") as ops_pool, \
             tc.tile_pool(name="bcpsum", bufs=1, space="PSUM") as bc_pool:
            for p in range(NPAIR):
                for ic in range(T // TC):
                    isl = slice(TC * ic, TC * (ic + 1))
                    oA = ops_pool.tile([65, TC], f32, name="oA")
                    oB = ops_pool.tile([65, TC], f32, name="oB")
                    nj = 4 * (ic + 1)
                    for j in range(nj):
                        jsl = slice(128 * j, 128 * (j + 1))
                        sp = sps_pool.tile([128, 2, TC], f32, name="sp")
                        nc.tensor.matmul(sp[:, 0, :], lhsT=kT[p][0:64, jsl],
                                         rhs=qT[p][0:64, isl],
                                         start=True, stop=True)
                        nc.tensor.matmul(sp[:, 1, :], lhsT=kT[p][64:128, jsl],
                                         rhs=qT[p][64:128, isl],
                                         start=True, stop=True)
                        pt = pt_pool.tile([128, 2, TC], bf16, name="ptt")
                        nc.scalar.activation(pt[:, :, :], sp[:, :, :],
                                             AF.Exp, scale=SCALE)
                        if j >= 4 * ic:
                            m = j - 4 * ic
                            nc.vector.tensor_mul(pt[:, 0, :], pt[:, 0, :],
                                                 masks[m][:, :])
                            nc.vector.tensor_mul(pt[:, 1, :], pt[:, 1, :],
                                                 masks[m][:, :])
                        st = (j == 0)
                        sp_last = (j == nj - 1)
                        nc.tensor.matmul(oA[:, :],
                                         lhsT=vts[j][:, 2 * p, 0:HD + 1],
                                         rhs=pt[:, 0, :],
                                         start=st, stop=sp_last)
                        nc.tensor.matmul(oB[:, :],
                                         lhsT=vts[j][:, 2 * p + 1, 0:HD + 1],
                                         rhs=pt[:, 1, :],
                                         start=st, stop=sp_last)
                    for half, o in ((0, oA), (1, oB)):
                        rf = rec_pool.tile([1, TC], f32, name="rf")
                        nc.vector.reciprocal(rf[:, :], o[64:65, :])
                        rb = rec_pool.tile([1, TC], bf16, name="rb")
                        nc.vector.tensor_copy(rb[:, :], rf[:, :])
                        bps = bc_pool.tile([HD, TC], f32, name="bps")
                        nc.tensor.matmul(bps[:, :], lhsT=ones_row[:, :],
                                         rhs=rb[:, :], start=True, stop=True)
                        bsb = sbbc_pool.tile([HD, TC], bf16, name="bsb")
                        nc.scalar.copy(bsb[:, :], bps[:, :])
                        nc.vector.tensor_mul(
                            outTn[p][64 * half:64 * (half + 1), isl],
                            o[0:HD, :], bsb[:, :])

        # ---------------- phase 3: output projection ----------------
        wo_pool = ctx.enter_context(tc.tile_pool(name="wo", bufs=1))
        wo_sb = []
        for f4 in range(NPAIR):
            t = wo_pool.tile([128, D], bf16, name=f"wo{f4}")
            nc.sync.dma_start(t[:, :], woT_d[128 * f4:128 * (f4 + 1), :])
            wo_sb.append(t)

        with tc.tile_pool(name="ypsum", bufs=2, space="PSUM") as yps_pool:
            for ot in range(D // 128):
                for ic in range(T // TC):
                    tsl = slice(TC * ic, TC * (ic + 1))
                    ps = yps_pool.tile([128, TC], f32, name="psy")
                    for f4 in range(NPAIR):
                        nc.tensor.matmul(
                            ps[:, :],
                            lhsT=wo_sb[f4][:, 128 * ot:128 * (ot + 1)],
                            rhs=outTn[f4][:, tsl],
                            start=(f4 == 0), stop=(f4 == NPAIR - 1),
                        )
                    yst = stage.tile([128, TC], f32, name="yst")
                    nc.scalar.copy(yst[:, :], ps[:, :])
                    nc.gpsimd.dma_start(
                        yT_d[128 * ot:128 * (ot + 1), tsl], yst[:, :])

    _split_waits(nc, mybir)
    return nc


class _Runner:
    """Compiles the bass program once and exposes a repeat-callable jitted
    executor over the 8 axon-attached NeuronCores."""

    def __init__(self):
        import jax
        import jax.numpy as jnp  # noqa: F401
        from jax.sharding import Mesh, PartitionSpec
        from jax.experimental.shard_map import shard_map
        from concourse import bass2jax, mybir

        os.environ.setdefault("JAX_COMPILATION_CACHE_DIR", "/tmp/jaxcache")
        try:
            jax.config.update("jax_compilation_cache_dir", "/tmp/jaxcache")
        except Exception:
            pass

        nc = _build_bass()
        bass2jax.install_neuronx_cc_hook()

        partition_name = (nc.partition_id_tensor.name
                          if nc.partition_id_tensor else None)
        in_names, out_names, out_avals = [], [], []
        for alloc in nc.m.functions[0].allocations:
            if not isinstance(alloc, mybir.MemoryLocationSet):
                continue
            name = alloc.memorylocations[0].name
            if alloc.kind == "ExternalInput":
                if name != partition_name:
                    in_names.append(name)
            elif alloc.kind == "ExternalOutput":
                out_names.append(name)
                out_avals.append(jax.core.ShapedArray(
                    tuple(alloc.tensor_shape), mybir.dt.np(alloc.dtype)))
        self.in_names = in_names
        self.out_names = out_names
        self.out_avals = out_avals

        zero_outs = [np.zeros(a.shape, a.dtype) for a in out_avals]

        bind_names = list(in_names) + list(out_names)
        if partition_name is not None:
            bind_names.append(partition_name)

        def _body(*args):
            operands = list(args)
            if partition_name is not None:
                operands.append(bass2jax.partition_id_tensor())
            outs = bass2jax._bass_exec_p.bind(
                *operands,
                out_avals=tuple(out_avals),
                in_names=tuple(bind_names),
                out_names=tuple(out_names),
                lowering_input_output_aliases=(),
                sim_require_finite=True,
                sim_require_nnan=True,
                nc=nc,
            )
            return tuple(outs)

        devices = jax.devices()[:NCORES]
        assert len(devices) == NCORES
        self.mesh = Mesh(np.asarray(devices), ("core",))
        spec = PartitionSpec("core")
        n_args = len(in_names) + len(out_names)
        self.fn = jax.jit(
            shard_map(_body, mesh=self.mesh,
                      in_specs=(spec,) * n_args,
                      out_specs=(spec,) * len(out_names),
                      check_rep=False),
            keep_unused=True,
        )
        self.zero_outs = zero_outs
        self.jax = jax

    def concat_inputs(self, in_maps):
        concat = [
            np.concatenate([np.asarray(m[name]) for m in in_maps], axis=0)
            for name in self.in_names
        ]
        concat += [
            np.zeros((NCORES * z.shape[0], *z.shape[1:]), z.dtype)
            for z in self.zero_outs
        ]
        return concat

    def __call__(self, in_maps):
        concat = self.concat_inputs(in_maps)
        outs = self.fn(*concat)
        res = []
        for c in range(NCORES):
            res.append({
                name: np.asarray(outs[i]).reshape(
                    NCORES, *self.out_avals[i].shape)[c]
                for i, name in enumerate(self.out_names)
            })
        return res

    def time_ns(self, in_maps, iters=10, warmup=2):
        """Steady-state wall-clock per call with device-resident inputs."""
        from jax.sharding import NamedSharding, PartitionSpec
        sh = NamedSharding(self.mesh, PartitionSpec("core"))
        concat = [self.jax.device_put(a, sh)
                  for a in self.concat_inputs(in_maps)]
        for _ in range(warmup):
            outs = self.fn(*concat)
            self.jax.block_until_ready(outs)
        times = []
        for _ in range(iters):
            t0 = time.perf_counter_ns()
            outs = self.fn(*concat)
            self.jax.block_until_ready(outs)
            times.append(time.perf_counter_ns() - t0)
        return min(times), times


def _get_runner():
    global _RUNNER
    if _RUNNER is None:
        _RUNNER = _Runner()
    return _RUNNER


def make_in_maps(inputs):
    import ml_dtypes
    bf16 = ml_dtypes.bfloat16
    x = np.asarray(inputs["x"], np.float32)
    Wq = np.asarray(inputs["Wq"], np.float32)
    Wk = np.asarray(inputs["Wk"], np.float32)
    Wv = np.asarray(inputs["Wv"], np.float32)
    Wo = np.asarray(inputs["Wo"], np.float32)
    bq = np.asarray(inputs["bq"], np.float32)
    bk = np.asarray(inputs["bk"], np.float32)
    bv = np.asarray(inputs["bv"], np.float32)

    in_maps = []
    for core in range(NCORES):
        b, g = core // 2, core % 2
        gs = slice(GF * g, GF * (g + 1))
        in_maps.append({
            "xT": np.ascontiguousarray(x[b].T).astype(bf16),
            "wqT": np.ascontiguousarray(Wq[gs, :].T).astype(bf16),
            "wkT": np.ascontiguousarray(Wk[gs, :].T).astype(bf16),
            "wvT": np.ascontiguousarray(Wv[gs, :].T).astype(bf16),
            "woT": np.ascontiguousarray(Wo[:, gs].T).astype(bf16),
            "bq": bq[gs].reshape(GF, 1).astype(np.float32),
            "bk": bk[gs].reshape(GF, 1).astype(np.float32),
            "bv": bv[gs].reshape(1, GF).astype(np.float32),
        })
    return in_maps


def assemble(results, bo):
    y = np.empty((B, T, D), np.float32)
    k = np.empty((B, H, T, HD), np.float32)
    v = np.empty((B, H, T, HD), np.float32)
    for b in range(B):
        yT = results[2 * b]["yT"] + results[2 * b + 1]["yT"]
        y[b] = yT.T + bo[None, :]
        for g in range(2):
            r = results[2 * b + g]
            k[b, 8 * g:8 * (g + 1)] = (
                r["kT"].reshape(8, HD, T).transpose(0, 2, 1))
            v[b, 8 * g:8 * (g + 1)] = (
                r["v"].reshape(T, 8, HD).transpose(1, 0, 2))
    return y, k, v


def kernel(**inputs):
    runner = _get_runner()
    in_maps = make_in_maps(inputs)
    results = runner(in_maps)
    bo = np.asarray(inputs["bo"], np.float32)
    return assemble(results, bo)
